# revision 1
# baseline (speedup 1.0000x reference)
"""Additive (Bahdanau) attention on Trainium2, 8 NeuronCores.

Work-balanced sharding: only key columns with k < valid_len contribute to the
output (masked columns underflow to exp(-1e6) = 0), so the host gathers the
valid (batch, k) columns, pads each batch's run to 32-column single-batch
chunks, and deals the chunks evenly across the 8 cores.  Each core computes,
for each of its chunks, the unnormalized partial attention output
sum_k exp(score)*values[k] and the partial softmax denominators sum_k
exp(score); the host sums partials per batch and normalizes.  With all keys
valid this degrades exactly to batch-per-core.

Per chunk (32 key columns of one batch) and h-tile t (h = 2x128):
  kT[h,w] = (W_k.T @ keysT_cols)        (PE, bf16)   [once per core]
  qT_c[h,q] = (W_q.T @ queriesT_chunk)  (PE, bf16)   [chunk's batch, host-placed]
  pre[h,(w,q)] = qT_c[h,q] + kT[h,w]    (DVE tensor_tensor 2x via pair-duplicated kt2)
  feat = tanh(pre) in place             (ACT - the bottleneck)
  scores[q,w] += feat.T @ w_v           (PE, one PSUM column per (w, qtile))
then per chunk: e = exp(scores+mask) (ACT, accum_out -> partial sums),
e.T (PE transpose), partial_out = e.T @ values_rows (PE), DMA partials out.

The chunk count per core is a compile-time constant; kernel() compiles/caches
one variant per needed count (1..8).
"""

import numpy as np

import concourse.bass as bass
import concourse.mybir as mybir
import concourse.tile as tile
from concourse import bacc
from concourse.bass_utils import run_bass_kernel_spmd

B, Q, K, H, D, DV = 8, 256, 256, 256, 256, 256
N_CORES = 8
F32 = mybir.dt.float32
BF16 = mybir.dt.bfloat16
AF = mybir.ActivationFunctionType
KC = 32  # key columns per chunk (single batch per chunk)


def build_nc(nchunks):
    W = KC * nchunks  # key columns per core
    # packed bf16 input columns: kTT(2*W) | wk(512) | wq(512) | qTT slots (nchunks*512)
    nbf = 2 * W + 1024 + nchunks * 512
    # packed f32 input columns: values rows (2*256) | mask (W) | identity (128) | wv (2)
    nf32 = 512 + W + 128 + 2
    nc = bacc.Bacc("TRN2", target_bir_lowering=False, name=f"addattn{nchunks}")
    d_bf = nc.dram_tensor("in_bf", [128, nbf], BF16, kind="ExternalInput")
    d_f = nc.dram_tensor("in_f32", [128, nf32], F32, kind="ExternalInput")
    # partial unnormalized outputs per (chunk, qtile), and partial sums
    d_outp = nc.dram_tensor("outp", [nchunks * 2, 128, DV], F32, kind="ExternalOutput")
    d_sums = nc.dram_tensor("sums", [2 * nchunks, 128], F32, kind="ExternalOutput")

    with tile.TileContext(nc) as tc:
        with (
            tc.tile_pool(name="sb", bufs=1) as sb,
            tc.tile_pool(name="feat", bufs=4) as feat_pool,
            tc.tile_pool(name="ps_scores", bufs=1, space=bass.MemorySpace.PSUM) as ps_s,
        ):
            # ------- packed inputs -------
            inbf = sb.tile([128, nbf], BF16, tag="inbf")
            nc.sync.dma_start(inbf[:], d_bf[:])
            inf = sb.tile([128, nf32], F32, tag="inf")
            nc.scalar.dma_start(inf[:], d_f[:])  # second HWDGE ring, runs in parallel
            kTT = [inbf[:, j * W:(j + 1) * W] for j in range(2)]
            wk_sb = [inbf[:, 2 * W + j * 256:2 * W + (j + 1) * 256] for j in range(2)]
            wq_sb = [inbf[:, 2 * W + 512 + j * 256:2 * W + 512 + (j + 1) * 256] for j in range(2)]
            qTTs = [[inbf[:, 2 * W + 1024 + c * 512 + j * 256:2 * W + 1024 + c * 512 + (j + 1) * 256]
                     for j in range(2)] for c in range(nchunks)]
            vals = [inf[:, t * 256:(t + 1) * 256] for t in range(2)]
            mask_sb = inf[:, 512:512 + W]
            ident = inf[:, 512 + W:512 + W + 128]
            wv_f = [inf[:, 512 + W + 128 + t:512 + W + 128 + t + 1] for t in range(2)]

            wv_b = [sb.tile([128, 1], BF16, tag=f"wvb{t}", name=f"wvb{t}") for t in range(2)]
            kT = [sb.tile([128, W], BF16, tag=f"kT{t}", name=f"kT{t}") for t in range(2)]
            kt2 = [sb.tile([128, 2 * W], BF16, tag=f"kt2{t}", name=f"kt2{t}") for t in range(2)]
            qT = [[sb.tile([128, Q], BF16, tag=f"qT{c}_{t}", name=f"qT{c}_{t}")
                   for t in range(2)] for c in range(nchunks)]
            s_ps = [[ps_s.tile([128, W], F32, tag=f"s{qt}_{t}", name=f"s{qt}_{t}")
                     for t in range(2)] for qt in range(2)]

            # ------- prep: projections (contract d); chunk-0/t=0 operands first so
            # the first main-loop add can start as early as possible -------
            with tc.tile_pool(name="ps_prep", bufs=2, space=bass.MemorySpace.PSUM) as ps_p:
                def proj_k(t):
                    nc.vector.tensor_copy(wv_b[t][:], wv_f[t])
                    pk = ps_p.tile([128, W], F32, tag="proj", name=f"pk{t}")
                    for j in range(2):
                        nc.tensor.matmul(pk[:], wk_sb[j][:, t * 128:(t + 1) * 128], kTT[j],
                                         start=(j == 0), stop=(j == 1))
                    nc.vector.tensor_copy(kT[t][:], pk[:])
                    nc.vector.tensor_copy(
                        kt2[t][:].rearrange("p (k e) -> p k e", e=2),
                        kT[t][:].unsqueeze(2).broadcast_to((128, W, 2)))

                def proj_q(c, t):
                    pq = ps_p.tile([128, 256], F32, tag="proj", name=f"pq{c}_{t}")
                    for j in range(2):
                        nc.tensor.matmul(pq[:], wq_sb[j][:, t * 128:(t + 1) * 128],
                                         qTTs[c][j], start=(j == 0), stop=(j == 1))
                    nc.vector.tensor_copy(qT[c][t][:], pq[:])

                proj_k(0)
                proj_q(0, 0)
                proj_k(1)
                proj_q(0, 1)
                for c in range(1, nchunks):
                    for t in range(2):
                        proj_q(c, t)

            # ------- main loop + per-chunk tail (emitted one chunk delayed so the
            # small exp instructions never stall the ACT FIFO behind pending MMs)
            ntile = (W + 127) // 128
            exp_sb = [sb.tile([128, W], F32, tag=f"exp{qt}", name=f"exp{qt}") for qt in range(2)]
            expT = [sb.tile([128, Q], F32, tag=f"expT{i}", name=f"expT{i}") for i in range(ntile)]
            sums_sb = sb.tile([128, 2 * nchunks], F32, tag="sums_sb")
            out_sb = sb.tile([128, 2 * nchunks * DV], F32, tag="out_sb")
            with tc.tile_pool(name="ps_tail", bufs=2, space=bass.MemorySpace.PSUM) as ps_t:

                def tail(c):
                    lo = c * KC
                    i, r = divmod(lo, 128)
                    for qt in range(2):
                        # TT may read only one PSUM operand: stage s1+mask into SBUF
                        nc.vector.tensor_add(exp_sb[qt][:, lo:lo + KC],
                                             s_ps[qt][1][:, lo:lo + KC],
                                             mask_sb[:, lo:lo + KC])
                        nc.vector.tensor_add(s_ps[qt][0][:, lo:lo + KC],
                                             s_ps[qt][0][:, lo:lo + KC],
                                             exp_sb[qt][:, lo:lo + KC])
                        nc.scalar.activation(
                            exp_sb[qt][:, lo:lo + KC],
                            s_ps[qt][0][:, lo:lo + KC], AF.Exp,
                            accum_out=sums_sb[:, qt * nchunks + c:qt * nchunks + c + 1])
                        tx = ps_t.tile([128, 128], F32, tag="tx")
                        nc.tensor.transpose(tx[:KC, :], exp_sb[qt][:, lo:lo + KC], ident)
                        nc.vector.tensor_copy(expT[i][r:r + KC, qt * 128:(qt + 1) * 128],
                                              tx[:KC, :])
                        av = ps_t.tile([128, DV], F32, tag="av")
                        nc.tensor.matmul(av[:], expT[i][r:r + KC, qt * 128:(qt + 1) * 128],
                                         vals[i][r:r + KC, :],
                                         start=True, stop=True, tile_position=(r, 0))
                        nc.vector.tensor_copy(
                            out_sb[:, (c * 2 + qt) * DV:(c * 2 + qt + 1) * DV], av[:])
                    nc.sync.dma_start(
                        d_outp[2 * c:2 * c + 2].transpose([1, 0, 2]),
                        out_sb[:, 2 * c * DV:(2 * c + 2) * DV].rearrange(
                            "p (g d) -> p g d", g=2))

                for c in range(nchunks):
                    k0 = c * KC
                    for t in range(2):
                        feat = feat_pool.tile([128, KC * Q], BF16, tag="feat")
                        # first-ever add+tanh is on the critical path: split it in
                        # halves so ACT starts sooner
                        nsub = 2 if (c == 0 and t == 0) else 1
                        sub = KC // nsub
                        for s in range(nsub):
                            j0 = s * sub
                            # pre[h,j,qp,e] = qT[h,2qp+e] + kT[h,k0+j]; pair APs keep 2x
                            in0 = qT[c][t][:].rearrange("p (qp e) -> p qp e", e=2)
                            in0 = in0.unsqueeze(1).broadcast_to((128, sub, Q // 2, 2))
                            in1 = kt2[t][:, 2 * (k0 + j0):2 * (k0 + j0 + sub)].rearrange(
                                "p (k e) -> p k e", e=2)
                            in1 = in1.unsqueeze(2).broadcast_to((128, sub, Q // 2, 2))
                            out = feat[:, j0 * Q:(j0 + sub) * Q].rearrange(
                                "p (a b c) -> p a b c", a=sub, b=Q // 2)
                            nc.vector.tensor_add(out, in0, in1)
                            nc.scalar.activation(feat[:, j0 * Q:(j0 + sub) * Q],
                                                 feat[:, j0 * Q:(j0 + sub) * Q], AF.Tanh)
                            for j in range(j0, j0 + sub):
                                w = k0 + j
                                for qt in range(2):
                                    nc.tensor.matmul(
                                        s_ps[qt][t][:, w:w + 1],
                                        feat[:, j * Q + qt * 128: j * Q + qt * 128 + 128],
                                        wv_b[t][:],
                                        start=True, stop=True)
                        if t == 0 and c > 0:
                            tail(c - 1)
                tail(nchunks - 1)
                nc.sync.dma_start(d_sums[:].transpose([1, 0]), sums_sb[:])
    nc.compile()
    return nc


_NCS = {}


def _get_nc(nchunks):
    if nchunks not in _NCS:
        _NCS[nchunks] = build_nc(nchunks)
    return _NCS[nchunks]


def _plan(valid_lens):
    """Global chunk list: each chunk = (batch, k0) covering keys [k0, k0+KC) of
    that batch (clipped to valid_len; padding columns masked)."""
    chunks = []
    for b in range(B):
        vl = min(max(int(valid_lens[b]), 0), K)
        for k0 in range(0, vl, KC):
            chunks.append((b, k0))
    nchunks = max(1, -(-len(chunks) // N_CORES))
    while len(chunks) < nchunks * N_CORES:
        chunks.append((-1, 0))  # dummy chunk
    return chunks, nchunks


def kernel(queries, keys, values, valid_lens, W_q, W_k, w_v):
    import ml_dtypes
    bf16 = ml_dtypes.bfloat16
    queries = np.asarray(queries, dtype=np.float32)
    keys = np.asarray(keys, dtype=np.float32)
    values = np.asarray(values, dtype=np.float32)
    valid_lens = np.asarray(valid_lens)
    W_q = np.asarray(W_q, dtype=np.float32)
    W_k = np.asarray(W_k, dtype=np.float32)
    w_v = np.asarray(w_v, dtype=np.float32).reshape(H)

    chunks, nchunks = _plan(valid_lens)
    nc = _get_nc(nchunks)
    W = KC * nchunks
    nbf = 2 * W + 1024 + nchunks * 512
    nf32 = 512 + W + 128 + 2

    wkb = W_k.astype(bf16)
    wqb = W_q.astype(bf16)
    ident = np.eye(128, dtype=np.float32)
    qTb = np.ascontiguousarray(np.transpose(queries, (0, 2, 1))).astype(bf16)  # [B, D, Q]
    kTb = np.ascontiguousarray(np.transpose(keys, (0, 2, 1))).astype(bf16)     # [B, D, K]

    in_maps = []
    core_chunks = []
    for cidx in range(N_CORES):
        my = chunks[cidx * nchunks:(cidx + 1) * nchunks]
        core_chunks.append(my)
        in_bf = np.zeros((128, nbf), dtype=bf16)
        in_f = np.zeros((128, nf32), dtype=np.float32)
        maskrow = np.full(W, -1.0e6, dtype=np.float32)
        for i, (b, k0) in enumerate(my):
            if b < 0:
                continue
            vl = int(valid_lens[b])
            n = min(KC, vl - k0)
            # keysT columns [D, n] and values rows [n, DV]
            kcols = kTb[b][:, k0:k0 + n]                      # [D, n]
            in_bf[:, i * KC:i * KC + n] = kcols[0:128]
            in_bf[:, W + i * KC:W + i * KC + n] = kcols[128:256]
            rows = values[b][k0:k0 + n]                       # [n, DV]
            lo = i * KC
            t0, r0 = divmod(lo, 128)
            in_f[r0:r0 + n, t0 * 256:(t0 + 1) * 256] = rows
            maskrow[lo:lo + n] = 0.0
            # qTT slot for this chunk
            in_bf[:, 2 * W + 1024 + i * 512:2 * W + 1024 + i * 512 + 256] = qTb[b][0:128]
            in_bf[:, 2 * W + 1024 + i * 512 + 256:2 * W + 1024 + i * 512 + 512] = qTb[b][128:256]
        in_bf[:, 2 * W:2 * W + 256] = wkb[0:128]
        in_bf[:, 2 * W + 256:2 * W + 512] = wkb[128:256]
        in_bf[:, 2 * W + 512:2 * W + 768] = wqb[0:128]
        in_bf[:, 2 * W + 768:2 * W + 1024] = wqb[128:256]
        in_f[:, 512:512 + W] = maskrow[None, :]
        in_f[:, 512 + W:512 + W + 128] = ident
        in_f[:, 512 + W + 128] = w_v[0:128]
        in_f[:, 512 + W + 129] = w_v[128:256]
        in_maps.append({"in_bf": in_bf, "in_f32": in_f})

    res = run_bass_kernel_spmd(nc, in_maps, core_ids=list(range(N_CORES)))
    return _combine(res.results, core_chunks, values, valid_lens, nchunks)


def _combine(results, core_chunks, values, valid_lens, nchunks):
    accum = np.zeros((B, Q, DV), dtype=np.float64)
    denom = np.zeros((B, Q), dtype=np.float64)
    for cidx in range(N_CORES):
        outp = results[cidx]["outp"].reshape(nchunks, 2, 128, DV)
        sums = results[cidx]["sums"].reshape(2, nchunks, 128)
        for i, (b, k0) in enumerate(core_chunks[cidx]):
            if b < 0:
                continue
            for qt in range(2):
                accum[b, qt * 128:(qt + 1) * 128] += outp[i, qt]
                denom[b, qt * 128:(qt + 1) * 128] += sums[qt, i]
    out = np.zeros((B, Q, DV), dtype=np.float32)
    for b in range(B):
        if int(valid_lens[b]) <= 0:
            # reference: softmax over all -1e6 scores is uniform
            out[b] = np.broadcast_to(values[b].mean(0), (Q, DV))
        else:
            out[b] = (accum[b] / denom[b][:, None]).astype(np.float32)
    return out


def run_spmd_traced(queries, keys, values, valid_lens, W_q, W_k, w_v, **kwargs):
    """test harness hook: same as kernel() but returns (output, BassKernelResults)."""
    import ml_dtypes  # noqa
    chunks, nchunks = _plan(np.asarray(valid_lens))
    # reuse kernel()'s packing by temporarily capturing run args
    global _LAST_RES
    res_holder = {}
    orig = run_bass_kernel_spmd

    def wrapper(nc, in_maps, core_ids, **kw):
        r = orig(nc, in_maps, core_ids=core_ids, **kw, **kwargs)
        res_holder["res"] = r
        return r

    g = globals()
    g["run_bass_kernel_spmd"] = wrapper
    try:
        out = kernel(queries, keys, values, valid_lens, W_q, W_k, w_v)
    finally:
        g["run_bass_kernel_spmd"] = orig
    return out, res_holder["res"]



# revision 7
# speedup vs baseline: 1.4958x; 1.4958x over previous
"""Additive (Bahdanau) attention on Trainium2, 8 NeuronCores.

Polynomial reformulation: tanh(x) on the needed range is replaced by an odd
degree-13 polynomial, and the (B,Q,K,H) feature tensor is never materialized:

  scores[q,w] = sum_h wv[h] * tanh(qp[h,q] + kp[h,w])
             ~= sum_{j+m=n odd} c_n*C(n,j) * sum_h (wv*qp^j)[h,q] * (kp^m)[h,w]

i.e. a sum of 56 (j,m) PE matmuls of bf16 power matrices, replacing ~64us of
ACT tanh + ~43us of DVE broadcast-adds per core with ~20us of dense PE MULTs.
Power chains are built on DVE (bf16); the per-pair coefficient-scaled K-power
tiles are split between DVE and the otherwise idle ACT engine.

Work-balanced sharding as before: only valid key columns (k < valid_len) are
computed; the host deals 32-key single-batch chunks across cores.  Chunks on
one core are grouped by batch ("groups"); the Q-side power chains are built
per group, K-side chains are shared.  Each core computes per-chunk partial
unnormalized outputs sum_w e[w,:]*V[w,:] plus the denominator sum_w e[w,:]
via a ones-column appended to V; the host sums partials per batch and
normalizes.

Per core: scores accumulate in PSUM [q=128, W] per q-half; mask-add (DVE) +
exp (ACT) -> e[q,W] f32; per chunk: PE transpose -> eT bf16, AV matmul
(eT[32,128] x [V|1][32,257]) -> partial out, DMA'd straight from PSUM.

Compiled variants are cached by the tuple of per-core group sizes.
"""

import math

import numpy as np

import concourse.bass as bass
import concourse.mybir as mybir
import concourse.tile as tile
from concourse import bacc
from concourse.bass_utils import run_bass_kernel_spmd

B, Q, K, H, D, DV = 8, 256, 256, 256, 256, 256
N_CORES = 8
F32 = mybir.dt.float32
BF16 = mybir.dt.bfloat16
AF = mybir.ActivationFunctionType
KC = 32  # key columns per chunk (single batch per chunk)
DEG = 13
NS = (1, 3, 5, 7, 9, 11, 13)
# weighted LS fit of tanh on [-5.3, 5.3], gaussian(std 0.813)+3e-3 floor
C_POLY = {1: 0.99121866, 3: -0.28722527, 5: 0.06722887, 7: -0.00871841,
          9: 0.00057647, 11: -1.825e-05, 13: 2.2e-07}


def _mlist(j):
    """m's paired with power j: j+m odd, j+m <= DEG."""
    return [m for m in range(DEG - j + 1) if (j + m) % 2 == 1]


def _coeff(j, m):
    return float(C_POLY[j + m] * math.comb(j + m, j))


def build_nc(gsizes):
    gsizes = tuple(gsizes)
    nchunks = sum(gsizes)
    G = len(gsizes)
    W = KC * nchunks
    VCB = -(-nchunks // 4)  # chunks packed 4 per 128 partitions
    # bf16 packed input columns
    QT_OFF = 0                       # G * 2 d-blocks * 256 (queriesT)
    KT_OFF = QT_OFF + G * 512        # 2 d-blocks * W (keysT cols)
    WQ_OFF = KT_OFF + 2 * W          # 4 x 128 (dp,hp blocks)
    WK_OFF = WQ_OFF + 512
    QP0_OFF = WK_OFF + 512           # 2 hp x 256 (wv broadcast)
    V_OFF = QP0_OFF + 512            # VCB x 257 ([V|1] rows)
    NBF = V_OFF + VCB * 257
    # f32 packed input columns: mask (W) | identity (128)
    NF = W + 128

    nc = bacc.Bacc("TRN2", target_bir_lowering=False,
                   name="paddattn" + "_".join(str(s) for s in gsizes))
    d_bf = nc.dram_tensor("in_bf", [128, NBF], BF16, kind="ExternalInput")
    d_f = nc.dram_tensor("in_f32", [128, NF], F32, kind="ExternalInput")
    d_outp = nc.dram_tensor("outp", [nchunks * 2, 128, 257], F32,
                            kind="ExternalOutput")

    with tile.TileContext(nc) as tc:
        with (
            tc.tile_pool(name="sb", bufs=1) as sb,
            tc.tile_pool(name="ps_s", bufs=1, space=bass.MemorySpace.PSUM) as ps_s,
        ):
            inbf = sb.tile([128, NBF], BF16, tag="inbf")
            nc.sync.dma_start(inbf[:], d_bf[:])
            inf = sb.tile([128, NF], F32, tag="inf")
            nc.scalar.dma_start(inf[:], d_f[:])

            qTT = [[inbf[:, QT_OFF + (g * 2 + dp) * 256:QT_OFF + (g * 2 + dp + 1) * 256]
                    for dp in range(2)] for g in range(G)]
            kTT = [inbf[:, KT_OFF + dp * W:KT_OFF + (dp + 1) * W] for dp in range(2)]
            wq = [[inbf[:, WQ_OFF + (dp * 2 + hp) * 128:WQ_OFF + (dp * 2 + hp + 1) * 128]
                   for hp in range(2)] for dp in range(2)]
            wk = [[inbf[:, WK_OFF + (dp * 2 + hp) * 128:WK_OFF + (dp * 2 + hp + 1) * 128]
                   for hp in range(2)] for dp in range(2)]
            qp0 = [inbf[:, QP0_OFF + hp * 256:QP0_OFF + (hp + 1) * 256] for hp in range(2)]
            vones = inbf[:, V_OFF:V_OFF + VCB * 257]
            mask_sb = inf[:, 0:W]
            ident = inf[:, W:W + 128]

            # ---- projections: qp[g][hp] [128,256] bf16, kp (=KP[1]) [128,W] ----
            qT = [[sb.tile([128, 256], BF16, tag=f"qT{g}_{hp}", name=f"qT{g}_{hp}")
                   for hp in range(2)] for g in range(G)]
            KP = [[None, None] for _ in range(DEG + 1)]  # KP[m][hp]
            for hp in range(2):
                KP[1][hp] = sb.tile([128, W], BF16, tag=f"KP1_{hp}", name=f"KP1_{hp}")
            with tc.tile_pool(name="ps_p", bufs=2, space=bass.MemorySpace.PSUM) as ps_p:
                for hp in range(2):
                    pk = ps_p.tile([128, W], F32, tag="proj", name=f"pk{hp}")
                    for dp in range(2):
                        nc.tensor.matmul(pk[:], wk[dp][hp], kTT[dp],
                                         start=(dp == 0), stop=(dp == 1))
                    nc.vector.tensor_copy(KP[1][hp][:], pk[:])
                for g in range(G):
                    for hp in range(2):
                        pq = ps_p.tile([128, 256], F32, tag="proj", name=f"pq{g}_{hp}")
                        for dp in range(2):
                            nc.tensor.matmul(pq[:], wq[dp][hp], qTT[g][dp],
                                             start=(dp == 0), stop=(dp == 1))
                        nc.vector.tensor_copy(qT[g][hp][:], pq[:])

            # ---- Q power chains (DVE): QP[g][j][hp] = wv * qp^j ----
            QP = [[[None, None] for _ in range(DEG + 1)] for _ in range(G)]
            for g in range(G):
                for hp in range(2):
                    QP[g][0][hp] = qp0[hp]
                for j in range(1, DEG + 1):
                    for hp in range(2):
                        t = sb.tile([128, 256], BF16, tag=f"QP{g}_{j}_{hp}",
                                    name=f"QP{g}_{j}_{hp}")
                        nc.vector.tensor_mul(t[:], QP[g][j - 1][hp][:] if j > 1
                                             else QP[g][0][hp], qT[g][hp][:])
                        QP[g][j][hp] = t

            # ---- K power chain + coefficient-scaled tiles (DVE/ACT/GPSIMD) ----
            # KS[j][m][hp] = coeff(j,m) * kp^m, emitted m-major so the j-descending
            # matmul order below can start as soon as early tiles exist.
            KS = {}
            need = [(j, m) for j in range(DEG + 1) for m in _mlist(j)]
            by_m = {}
            for j, m in need:
                by_m.setdefault(m, []).append(j)
            alt = 0
            for m in range(DEG + 1):
                if m >= 2:
                    for hp in range(2):
                        t = sb.tile([128, W], BF16, tag=f"KP{m}_{hp}",
                                    name=f"KP{m}_{hp}")
                        nc.vector.tensor_mul(t[:], KP[m - 1][hp][:], KP[1][hp][:])
                        KP[m][hp] = t
                for j in sorted(by_m.get(m, []), reverse=True):
                    c = _coeff(j, m)
                    for hp in range(2):
                        t = sb.tile([128, W], BF16, tag=f"KS{j}_{m}_{hp}",
                                    name=f"KS{j}_{m}_{hp}")
                        if m == 0:
                            nc.gpsimd.memset(t[:], c)
                        else:
                            # split scale work between DVE and the idle ACT
                            if alt % 3 == 2:
                                nc.scalar.mul(t[:], KP[m][hp][:], c)
                            else:
                                nc.vector.tensor_scalar_mul(t[:], KP[m][hp][:], c)
                            alt += 1
                        KS[(j, m, hp)] = t

            # ---- score matmuls: s_ps[qt][q, W] += QP_j^T @ KS_jm per group ----
            s_ps = [ps_s.tile([128, W], F32, tag=f"s{qt}", name=f"s{qt}")
                    for qt in range(2)]
            goff = []
            o = 0
            for g in range(G):
                goff.append(o)
                o += KC * gsizes[g]
            e_f = [sb.tile([128, W], F32, tag=f"e{qt}", name=f"e{qt}")
                   for qt in range(2)]

            def scores(qt):
                for g in range(G):
                    lo, wg = goff[g], KC * gsizes[g]
                    seq = [(hp, j, m) for hp in range(2)
                           for j in range(DEG, -1, -1) for m in _mlist(j)]
                    for i, (hp, j, m) in enumerate(seq):
                        nc.tensor.matmul(
                            s_ps[qt][:, lo:lo + wg],
                            QP[g][j][hp][:, qt * 128:(qt + 1) * 128]
                            if j > 0 else qp0[hp][:, qt * 128:(qt + 1) * 128],
                            KS[(j, m, hp)][:, lo:lo + wg],
                            start=(i == 0), stop=(i == len(seq) - 1))

            def exp_qt(qt):
                nc.vector.tensor_add(e_f[qt][:], s_ps[qt][:], mask_sb)
                nc.scalar.activation(e_f[qt][:], e_f[qt][:], AF.Exp)

            scores(0)
            exp_qt(0)
            scores(1)
            exp_qt(1)

            # ---- per-chunk tail: transpose e -> eT bf16, AV matmul, DMA out ----
            eT = sb.tile([128, VCB * 256], BF16, tag="eT")
            out_sb = sb.tile([128, 2 * nchunks * 257], F32, tag="out_sb")
            with tc.tile_pool(name="ps_t", bufs=3, space=bass.MemorySpace.PSUM) as ps_t:
                for qt in range(2):
                    for c in range(nchunks):
                        r, cb = KC * (c % 4), c // 4
                        tx = ps_t.tile([32, 128], F32, tag="tx")
                        nc.tensor.transpose(tx[:], e_f[qt][:, KC * c:KC * (c + 1)],
                                            ident)
                        nc.vector.tensor_copy(
                            eT[r:r + KC, cb * 256 + qt * 128:cb * 256 + (qt + 1) * 128],
                            tx[:])
                        av = ps_t.tile([128, 257], F32, tag="av")
                        nc.tensor.matmul(
                            av[:],
                            eT[r:r + KC, cb * 256 + qt * 128:cb * 256 + (qt + 1) * 128],
                            vones[r:r + KC, cb * 257:(cb + 1) * 257],
                            start=True, stop=True, tile_position=(r, 0))
                        o = (2 * c + qt) * 257
                        nc.scalar.copy(out_sb[:, o:o + 257], av[:])
                        nc.sync.dma_start(d_outp[2 * c + qt], out_sb[:, o:o + 257])
    nc.compile()
    return nc


_NCS = {}


def _get_nc(gsizes):
    gsizes = tuple(gsizes)
    if gsizes not in _NCS:
        _NCS[gsizes] = build_nc(gsizes)
    return _NCS[gsizes]


def _plan(valid_lens):
    """Deal valid-key chunks into a UNIFORM per-core group structure.

    Every core gets group slots of sizes (ceil(n/2), floor(n/2)) chunks, each
    slot single-batch (padded with dummy chunks where needed), so one compiled
    kernel variant serves all 8 cores in a single SPMD launch.

    Returns (core_plans, nchunks): core_plans[i] = (chunks, groups) with
    chunks = [(b, k0)] in packed order, groups = [(qbatch, slot_size)].
    """
    runs = []
    for b in range(B):
        vl = min(max(int(valid_lens[b]), 0), K)
        ks = list(range(0, vl, KC))
        if ks:
            runs.append([b, ks])
    total = sum(len(ks) for _, ks in runs)
    runs.sort(key=lambda r: -len(r[1]))

    def solve(nchunks):
        """Backtracking: split each batch run into (k1 x s1, k2 x s2) slot
        fills with total padding <= dummy budget.  Returns list of
        (slot_size, k1, k2) per run, or None."""
        s1, s2 = nchunks - nchunks // 2, nchunks // 2
        budget = nchunks * N_CORES - total
        sizes = [len(ks) for _, ks in runs]

        def rec(i, c1, c2, slack):
            if i == len(sizes):
                return []
            n = sizes[i]
            cands = []
            for k1 in range(min(c1, -(-n // s1)) + 1):
                rem = n - k1 * s1
                k2min = 0 if rem <= 0 else (-(-rem // s2) if s2 else None)
                if k2min is None or (s2 and k2min > c2):
                    continue
                cap = k1 * s1 + k2min * (s2 or 0)
                if cap < n:
                    continue
                cands.append((cap - n, k1, k2min))
            cands.sort()
            for sl, k1, k2 in cands:
                if slack + sl > budget:
                    continue
                sub = rec(i + 1, c1 - k1, c2 - k2, slack + sl)
                if sub is not None:
                    return [(k1, k2)] + sub
            return None

        sol = rec(0, N_CORES, N_CORES if s2 else 0, 0)
        return sol, s1, s2

    nchunks = max(1, -(-total // N_CORES))
    for _ in range(8):
        sol, s1, s2 = solve(nchunks)
        if sol is not None:
            break
        nchunks += 1
    assert sol is not None, "uniform slot packing failed"
    navail = [N_CORES, N_CORES if s2 else 0]
    by_type = [[], []]  # slot_type -> [(batch, [k0...])]
    for (b, ks), (k1, k2) in zip(runs, sol):
        pos = 0
        for ty, s, cnt in ((0, s1, k1), (1, s2, k2)):
            for _ in range(cnt):
                by_type[ty].append((b, ks[pos:pos + s]))
                pos += s
                navail[ty] -= 1
    # leftover slots become all-dummy groups (batch 0 for the q side)
    for ty in (0, 1):
        for _ in range(navail[ty]):
            by_type[ty].append((0, []))
    core_plans = []
    for i in range(N_CORES):
        groups, chunks = [], []
        for ty, s in ([(0, s1), (1, s2)] if s2 else [(0, s1)]):
            b, real = by_type[ty].pop()
            groups.append((b, s))
            chunks.extend((b, k0) for k0 in real)
            chunks.extend((-1, 0) for _ in range(s - len(real)))
        core_plans.append((chunks, groups))
    return core_plans, nchunks


def kernel(queries, keys, values, valid_lens, W_q, W_k, w_v):
    import ml_dtypes
    bf16 = ml_dtypes.bfloat16
    queries = np.asarray(queries, dtype=np.float32)
    keys = np.asarray(keys, dtype=np.float32)
    values = np.asarray(values, dtype=np.float32)
    valid_lens = np.asarray(valid_lens)
    W_q = np.asarray(W_q, dtype=np.float32)
    W_k = np.asarray(W_k, dtype=np.float32)
    w_v = np.asarray(w_v, dtype=np.float32).reshape(H)

    core_plans, nchunks = _plan(valid_lens)
    W = KC * nchunks
    VCB = -(-nchunks // 4)

    wqb = W_q.astype(bf16)   # [D, H]
    wkb = W_k.astype(bf16)
    wvb = w_v.astype(bf16)
    ident = np.eye(128, dtype=np.float32)
    qTb = np.ascontiguousarray(np.transpose(queries, (0, 2, 1))).astype(bf16)
    kTb = np.ascontiguousarray(np.transpose(keys, (0, 2, 1))).astype(bf16)
    vb = values.astype(bf16)

    in_maps = []
    gsizes_per_core = []
    for cidx in range(N_CORES):
        chunks, groups = core_plans[cidx]
        gsizes = tuple(s for _, s in groups)
        gsizes_per_core.append(gsizes)
        G = len(gsizes)
        QT_OFF = 0
        KT_OFF = QT_OFF + G * 512
        WQ_OFF = KT_OFF + 2 * W
        WK_OFF = WQ_OFF + 512
        QP0_OFF = WK_OFF + 512
        V_OFF = QP0_OFF + 512
        NBF = V_OFF + VCB * 257
        NF = W + 128

        in_bf = np.zeros((128, NBF), dtype=bf16)
        in_f = np.zeros((128, NF), dtype=np.float32)
        maskrow = np.full(W, -1.0e6, dtype=np.float32)
        for g, (gb, _) in enumerate(groups):
            for dp in range(2):
                in_bf[:, QT_OFF + (g * 2 + dp) * 256:QT_OFF + (g * 2 + dp + 1) * 256] = \
                    qTb[gb][dp * 128:(dp + 1) * 128]
        for i, (b, k0) in enumerate(chunks):
            if b < 0:
                continue
            vl = int(valid_lens[b])
            n = min(KC, vl - k0)
            kcols = kTb[b][:, k0:k0 + n]
            for dp in range(2):
                in_bf[:, KT_OFF + dp * W + i * KC:KT_OFF + dp * W + i * KC + n] = \
                    kcols[dp * 128:(dp + 1) * 128]
            maskrow[i * KC:i * KC + n] = 0.0
            r, cb = KC * (i % 4), i // 4
            in_bf[r:r + n, V_OFF + cb * 257:V_OFF + cb * 257 + 256] = vb[b][k0:k0 + n]
            in_bf[r:r + n, V_OFF + cb * 257 + 256] = 1.0
        for dp in range(2):
            for hp in range(2):
                in_bf[:, WQ_OFF + (dp * 2 + hp) * 128:WQ_OFF + (dp * 2 + hp + 1) * 128] = \
                    wqb[dp * 128:(dp + 1) * 128, hp * 128:(hp + 1) * 128]
                in_bf[:, WK_OFF + (dp * 2 + hp) * 128:WK_OFF + (dp * 2 + hp + 1) * 128] = \
                    wkb[dp * 128:(dp + 1) * 128, hp * 128:(hp + 1) * 128]
        for hp in range(2):
            in_bf[:, QP0_OFF + hp * 256:QP0_OFF + (hp + 1) * 256] = \
                np.broadcast_to(wvb[hp * 128:(hp + 1) * 128, None], (128, 256))
        in_f[:, 0:W] = maskrow[None, :]
        in_f[:, W:W + 128] = ident
        in_maps.append({"in_bf": in_bf, "in_f32": in_f})

    # compile all needed variants, then run
    for gs in set(gsizes_per_core):
        _get_nc(gs)
    if len(set(gsizes_per_core)) == 1:
        nc = _get_nc(gsizes_per_core[0])
        res = run_bass_kernel_spmd(nc, in_maps, core_ids=list(range(N_CORES)))
        results = res.results
    else:
        # run homogeneous subsets per variant
        results = [None] * N_CORES
        for gs in sorted(set(gsizes_per_core)):
            ids = [i for i in range(N_CORES) if gsizes_per_core[i] == gs]
            nc = _get_nc(gs)
            res = run_bass_kernel_spmd(nc, [in_maps[i] for i in ids], core_ids=ids)
            for i, r in zip(ids, res.results):
                results[i] = r
    return _combine(results, core_plans, values, valid_lens, nchunks)


def _combine(results, core_plans, values, valid_lens, nchunks):
    accum = np.zeros((B, Q, DV), dtype=np.float64)
    denom = np.zeros((B, Q), dtype=np.float64)
    for cidx in range(N_CORES):
        outp = results[cidx]["outp"].reshape(nchunks, 2, 128, 257)
        chunks, _ = core_plans[cidx]
        for i, (b, k0) in enumerate(chunks):
            if b < 0:
                continue
            for qt in range(2):
                accum[b, qt * 128:(qt + 1) * 128] += outp[i, qt][:, :256]
                denom[b, qt * 128:(qt + 1) * 128] += outp[i, qt][:, 256]
    out = np.zeros((B, Q, DV), dtype=np.float32)
    for b in range(B):
        if int(valid_lens[b]) <= 0:
            out[b] = np.broadcast_to(values[b].mean(0), (Q, DV))
        else:
            out[b] = (accum[b] / denom[b][:, None]).astype(np.float32)
    return out


def run_spmd_traced(queries, keys, values, valid_lens, W_q, W_k, w_v, **kwargs):
    """test harness hook: same as kernel() but returns (output, BassKernelResults)."""
    res_holder = {}
    orig = run_bass_kernel_spmd

    def wrapper(nc, in_maps, core_ids, **kw):
        r = orig(nc, in_maps, core_ids=core_ids, **kw, **kwargs)
        if "res" not in res_holder:
            res_holder["res"] = r
        else:  # multiple variants: keep the max exec time
            prev = res_holder["res"]
            if (r.exec_time_ns or 0) > (prev.exec_time_ns or 0):
                res_holder["res"] = r
        return r

    g = globals()
    g["run_bass_kernel_spmd"] = wrapper
    try:
        out = kernel(queries, keys, values, valid_lens, W_q, W_k, w_v)
    finally:
        g["run_bass_kernel_spmd"] = orig
    return out, res_holder["res"]
